# revision 1
# baseline (speedup 1.0000x reference)
"""APPNP GNN kernel for 8 Trainium2 NeuronCores.

Per core (nodes sharded, 12672 lanes incl. pads):
  MLP: X @ W0.T -> relu -> @ W1.T -> relu -> @ W2.T  (bf16 matmuls, f32 psum)
  K=10 propagation steps:
    hs = h * norm_src                      (bf16, [128, NT, 64])
    AllGather hs -> hbuf [NPAD, 64] bf16   (viewed as pair-table [NPAD/2, 128])
    dma_gather per (chunk, seg, half): 128B rows from the 256B-strided table
    one-hot S tiles (DVE is_equal vs iota) + S-stationary matmul segment-sum
    h = (1-a)*norm_dst*agg + a*h0
"""
import sys

sys.path.insert(0, "/opt/trn_rl_repo")

import numpy as np
import ml_dtypes

import inspect
import textwrap

import concourse.bass as bass
import concourse.bacc as bacc
import concourse.tile as tile
import concourse.mybir as mybir
from concourse.bass_utils import run_bass_kernel_spmd
from concourse.alu_op_type import AluOpType

BF16 = ml_dtypes.bfloat16
F32 = mybir.dt.float32
BF = mybir.dt.bfloat16
I16 = mybir.dt.int16

# problem constants
N = 100000
E = 1000000
IN = 512
C = 64
K = 10
ALPHA = 0.1

NCORES = 8
SH_N = N // NCORES              # real nodes per core
NTILES = (SH_N + 127) // 128 + 1  # node tiles per core (+1 all-pad tile)
SH = NTILES * 128               # padded nodes per core
NPAD = NCORES * SH
GQUEUES = 4                     # SWDGE queues for gather round-robin
CHUNK_BANKS = 16                # banks per gather chunk
PAD_LANE_VAL = 200.0            # dst-lane sentinel for pad slots

# split-collective: tiles [0, TH) -> table A, [TH, NTILES) -> table B.
# Each table AllGathers separately (double-buffered) so the collective for
# step s+1 overlaps step s's tail / step s+1's gathers on the other table.
TH = NTILES // 2                   # 49
TB = NTILES - TH                   # 50 (incl pad tile)
PAIRS_A = NCORES * 128 * TH // 2   # pair rows in table A (< 32768)
PAIRS_B = NCORES * 128 * TB // 2
assert PAIRS_A < 32768 and PAIRS_B < 32768


def _install_dma_gather_patched():
    """Clone bass dma_gather with the 256B elem-size assert relaxed to 128B.

    The underlying ucode supports 128B transfers over a 256B-strided table;
    only the bass-side assert (written for the transpose path) blocks it.
    """
    if hasattr(bass.BassGpSimd, "dma_gather_patched"):
        return bass.BassGpSimd.dma_gather_patched
    src = inspect.getsource(bass.BassGpSimd.dma_gather)
    src = textwrap.dedent(src)
    src = src.replace(
        "elem_size_bytes > 0 and elem_size_bytes % 256 == 0",
        "elem_size_bytes > 0 and elem_size_bytes % 64 == 0",
    )
    src = src.replace("def dma_gather(", "def dma_gather_patched(")
    ns = dict(bass.BassGpSimd.dma_gather.__globals__)
    exec(compile(src, "<dma_gather_patched>", "exec"), ns)
    fn = ns["dma_gather_patched"]
    bass.BassGpSimd.dma_gather_patched = fn
    return fn


# ---------------------------------------------------------------------------
# host-side graph preprocessing
# ---------------------------------------------------------------------------

def preprocess(edge_index):
    src = np.asarray(edge_index[0], dtype=np.int64)
    dst = np.asarray(edge_index[1], dtype=np.int64)
    ne = src.shape[0]
    deg_out = np.bincount(src, minlength=N).astype(np.float32)
    deg_in = np.bincount(dst, minlength=N).astype(np.float32)
    ns_full = np.maximum(deg_out, 1.0) ** -0.5
    nd_full = np.maximum(deg_in, 1.0) ** -0.5

    core_of = np.arange(N) // SH_N
    lane_of = np.arange(N) % SH_N          # natural order within core
    p_of = lane_of % 128
    t_of = lane_of // 128
    # per-table row id (matches [128, TH/TB, 64] (p, t, e) DMA layout)
    in_b = t_of >= TH
    rr_of = np.where(
        in_b,
        core_of * (128 * TB) + p_of * TB + (t_of - TH),
        core_of * (128 * TH) + p_of * TH + t_of)

    nb = NTILES - 1  # active banks

    # edge -> (core, bank, class); class = table * 2 + half-of-pair-row
    c_e = core_of[dst]
    nt_e = t_of[dst]
    rr_s = rr_of[src]
    kappa_e = in_b[src] * 2 + (rr_s % 2)  # class 0..3

    # counts per (core, bank, class)
    key = ((c_e * nb + nt_e) * 4 + kappa_e).astype(np.int64)
    cnt = np.bincount(key, minlength=NCORES * nb * 4).reshape(NCORES, nb, 4)
    tk = -(-cnt // 128)              # tiles per (core, bank, class)
    TK = tk.max(axis=0)              # common schedule [nb, 4]

    # chunk structure over banks
    chunk_starts = list(range(0, nb, CHUNK_BANKS))
    chunks = [(s, min(s + CHUNK_BANKS, nb)) for s in chunk_starts]

    # column layout: for chunk: for kappa: for nt in chunk: TK[nt, kappa] tiles
    col_of = np.zeros((nb, 4), np.int64)   # first col of (nt, kappa)
    call_cols = []                          # per (chunk, kappa): (col0, ncols)
    col = 0
    for (b0, b1) in chunks:
        for kap in range(4):
            c0 = col
            for nt in range(b0, b1):
                col_of[nt, kap] = col
                col += TK[nt, kap]
            call_cols.append((c0, col - c0))
    ncols = col

    # pad target rows per class: pad slots carry dl=PAD_LANE_VAL, so their
    # S column is all-zero and the (finite, real) gathered row contributes
    # nothing -- any in-range row works; use row 0 of each table.
    pad_pair_local = np.zeros(4, np.int64)

    # edge sort: by (core, bank, class)
    order_e = np.argsort(key, kind="stable")
    key_s = key[order_e]
    starts = np.concatenate([[0], np.cumsum(np.bincount(
        key_s, minlength=NCORES * nb * 4))])
    pos_e = np.arange(ne) - starts[key_s]

    src_sorted = src[order_e]
    dst_sorted = dst[order_e]
    c_s = c_e[order_e]
    nt_s = nt_e[order_e]
    kap_s = kappa_e[order_e]
    rr_ss = rr_of[src_sorted]
    pair_local_s = rr_ss // 2
    assert pair_local_s.max() < 32768
    lane_s = p_of[dst_sorted].astype(np.float32)

    gcol_s = col_of[nt_s, kap_s] + pos_e // 128
    gp_s = pos_e % 128

    colclass = np.empty(ncols, np.int64)
    for nt in range(nb):
        for kap in range(4):
            colclass[col_of[nt, kap]:col_of[nt, kap] + TK[nt, kap]] = kap

    idx_arrs = []   # [128, ncols] int16 (slot s at partition s)
    dl_arrs = []    # [128, ncols] f32 dst-lane per slot
    for c in range(NCORES):
        m = c_s == c
        idx_a = np.empty((128, ncols), np.int16)
        dl_a = np.full((128, ncols), PAD_LANE_VAL, np.float32)
        idx_a[:] = pad_pair_local[colclass][None, :].astype(np.int16)
        idx_a[gp_s[m], gcol_s[m]] = pair_local_s[m].astype(np.int16)
        dl_a[gp_s[m], gcol_s[m]] = lane_s[m]
        idx_arrs.append(idx_a)
        dl_arrs.append(dl_a)

    # wrapped gather index inputs: per call block [128, n_call/16]
    call_meta = []  # (kappa, col0, ncols_call, wrap_col0)
    wrap_col = 0
    for ci, (b0, b1) in enumerate(chunks):
        for kap in range(4):
            c0, ncol = call_cols[ci * 4 + kap]
            call_meta.append((kap, c0, ncol, wrap_col))
            wrap_col += ncol * 8  # 128 idx per col -> 8 wrap-cols
    totw = wrap_col
    idxw_arrs = []
    for c in range(NCORES):
        w = np.zeros((128, totw), np.int16)
        for (kap, c0, ncol, w0) in call_meta:
            if ncol == 0:
                continue
            blk = idx_arrs[c][:, c0:c0 + ncol]          # [128, ncol]
            flat = blk.T.reshape(-1)                    # slot order (col-major)
            wrapped = flat.reshape(-1, 16).T            # [16, ncol*8]
            w[:, w0:w0 + ncol * 8] = np.tile(wrapped, (8, 1))
        idxw_arrs.append(w)

    # norms [128, NTILES]
    ns_arrs, nd_arrs = [], []
    for c in range(NCORES):
        ns_a = np.zeros((128, NTILES), np.float32)
        nd_a = np.zeros((128, NTILES), np.float32)
        nodes = np.arange(c * SH_N, (c + 1) * SH_N)
        ns_a[p_of[nodes], t_of[nodes]] = ns_full[nodes]
        nd_a[p_of[nodes], t_of[nodes]] = (1.0 - ALPHA) * nd_full[nodes]
        ns_arrs.append(ns_a)
        nd_arrs.append(nd_a)

    iota = np.tile(np.arange(128, dtype=np.float32)[None, :], (128, 1))

    meta = dict(TK=TK, col_of=col_of, chunks=chunks, call_meta=call_meta,
                ncols=ncols, totw=totw)
    return dict(meta=meta, idxw_arrs=idxw_arrs, dl_arrs=dl_arrs,
                ns_arrs=ns_arrs, nd_arrs=nd_arrs, iota=iota,
                core_of=core_of, p_of=p_of, t_of=t_of)


# ---------------------------------------------------------------------------
# device graph builder
# ---------------------------------------------------------------------------

def build(meta, skip_cc=False, skip_gather=False, skip_mm=False, gqueues=None,
          gather_elem=None, gcall=8):
    if gqueues is None:
        gqueues = GQUEUES
    """gather_elem: timing-only experiment — memset msg tiles (as skip_gather)
    and issue gathers of gather_elem elements into a scratch tile instead.
    Results are wrong; only the schedule/timing is meaningful."""
    import os
    dma_gather_p = _install_dma_gather_patched()

    TK = meta["TK"]
    col_of = meta["col_of"]
    chunks = meta["chunks"]
    call_meta = meta["call_meta"]
    ncols = meta["ncols"]
    totw = meta["totw"]
    nb = NTILES - 1

    nc = bacc.Bacc("TRN2", target_bir_lowering=False, debug=False,
                   num_devices=NCORES, num_swdge_queues=gqueues)

    xt = nc.dram_tensor("xt", [128, 4, SH], BF, kind="ExternalInput")
    w0t = nc.dram_tensor("w0t", [128, 4, 512], BF, kind="ExternalInput")
    w1t = nc.dram_tensor("w1t", [128, 4, 256], BF, kind="ExternalInput")
    w2t = nc.dram_tensor("w2t", [128, 2, 64], BF, kind="ExternalInput")
    b0c = nc.dram_tensor("b0c", [128, 4], F32, kind="ExternalInput")
    b1c = nc.dram_tensor("b1c", [128, 2], F32, kind="ExternalInput")
    b2r = nc.dram_tensor("b2r", [128, 64], F32, kind="ExternalInput")
    nsb = nc.dram_tensor("nsb", [128, NTILES], F32, kind="ExternalInput")
    ndb = nc.dram_tensor("ndb", [128, NTILES], F32, kind="ExternalInput")
    iod = nc.dram_tensor("iota", [128, 128], F32, kind="ExternalInput")
    idxd = nc.dram_tensor("idxw", [128, totw], I16, kind="ExternalInput")
    dld = nc.dram_tensor("dlane", [128, ncols], F32, kind="ExternalInput")
    out = nc.dram_tensor("out", [128, NTILES, 64], F32, kind="ExternalOutput")

    MAXTK = int(TK.max())
    max_chunk_cols = max(
        int(sum(TK[nt, kap] for nt in range(b0, b1) for kap in range(4)))
        for (b0, b1) in chunks)

    with tile.TileContext(nc) as tc:
        with (
            tc.tile_pool(name="const", bufs=1) as constp,
            tc.tile_pool(name="state", bufs=1) as statep,
            tc.tile_pool(name="xtp", bufs=3) as xtp,
            tc.tile_pool(name="a1p", bufs=2) as a1p,
            tc.tile_pool(name="a2p", bufs=2) as a2p,
            tc.tile_pool(name="msgp", bufs=2) as msgp,
            tc.tile_pool(name="sp", bufs=3) as spool,
            tc.tile_pool(name="psw", bufs=2, space="PSUM") as ps_w,
            tc.tile_pool(name="psn", bufs=4, space="PSUM") as ps_n,
            tc.tile_pool(name="dram", bufs=1, space="DRAM") as dramp,
        ):
            # ---- constants ----
            w0s = constp.tile([128, 4, 512], BF)
            w1s = constp.tile([128, 4, 256], BF)
            w2s = constp.tile([128, 2, 64], BF)
            b0s = constp.tile([128, 4], F32)
            b1s = constp.tile([128, 2], F32)
            b2s = constp.tile([128, 64], F32)
            nss = constp.tile([128, NTILES], F32)
            nds = constp.tile([128, NTILES], F32)
            ios = constp.tile([128, 128], F32)
            idxs = constp.tile([128, totw], I16)
            dls = constp.tile([128, ncols], F32)
            for dst_t, src_t in [(w0s, w0t), (w1s, w1t), (w2s, w2t),
                                 (b0s, b0c), (b1s, b1c), (b2s, b2r),
                                 (nss, nsb), (nds, ndb), (ios, iod),
                                 (idxs, idxd), (dls, dld)]:
                nc.sync.dma_start(dst_t[:], src_t[:])

            h_cur = statep.tile([128, NTILES, 64], F32)
            h0a = statep.tile([128, NTILES, 64], F32)
            hs = statep.tile([128, NTILES, 64], BF)

            # pad bank: zero once
            nc.vector.memset(h_cur[:, nb, :], 0.0)
            nc.vector.memset(h0a[:, nb, :], 0.0)
            nc.vector.memset(hs[:, nb, :], 0.0)

            # split-collective tables, double-buffered so the AllGather for
            # step s+1 overlaps step s's tail and step s+1's other-table work
            ccA = [dramp.tile([128, TH, 64], BF, name=f"ccA{i}")
                   for i in range(2)]
            ccB = [dramp.tile([128, TB, 64], BF, name=f"ccB{i}")
                   for i in range(2)]
            hbA = [dramp.tile([PAIRS_A, 128], BF, name=f"hbA{i}")
                   for i in range(2)]
            hbB = [dramp.tile([PAIRS_B, 128], BF, name=f"hbB{i}")
                   for i in range(2)]
            gq_ctr = [0]

            def emit_cc(buf, pp):
                cin = ccA[pp] if buf == 0 else ccB[pp]
                hb = hbA[pp] if buf == 0 else hbB[pp]
                sl = slice(0, TH) if buf == 0 else slice(TH, NTILES)
                nc.sync.dma_start(cin[:], hs[:, sl, :])
                if not skip_cc:
                    nc.gpsimd.collective_compute(
                        "AllGather",
                        mybir.AluOpType.bypass,
                        replica_groups=[list(range(NCORES))],
                        ins=[cin.opt()],
                        outs=[hb.opt()],
                    )

            # last MLP chunk whose hs tiles complete table A
            mlp_a_chunk = (TH - 1) // 2
            cc_a_chunk = (TH - 1) // CHUNK_BANKS  # prop chunk completing A

            # ---- MLP over chunks of 2 node-tiles ----
            for ch in range(nb // 2):
                c0 = ch * 256
                xt_t = xtp.tile([128, 4, 256], BF)
                nc.sync.dma_start(xt_t[:], xt[:, :, c0:c0 + 256])
                a1_t = a1p.tile([128, 4, 256], BF)
                for mt in range(4):
                    psw_t = ps_w.tile([128, 256], F32)
                    ps1 = psw_t
                    for k in range(4):
                        nc.tensor.matmul(
                            ps1[:],
                            lhsT=w0s[:, k, mt * 128:(mt + 1) * 128],
                            rhs=xt_t[:, k, :],
                            start=(k == 0), stop=(k == 3),
                        )
                    nc.scalar.activation(
                        a1_t[:, mt, :], ps1[:],
                        mybir.ActivationFunctionType.Relu,
                        bias=b0s[:, mt:mt + 1],
                    )
                a2_t = a2p.tile([128, 2, 256], BF)
                for mt in range(2):
                    psw_t = ps_w.tile([128, 256], F32)
                    ps2 = psw_t
                    for k in range(4):
                        nc.tensor.matmul(
                            ps2[:],
                            lhsT=w1s[:, k, mt * 128:(mt + 1) * 128],
                            rhs=a1_t[:, k, :],
                            start=(k == 0), stop=(k == 3),
                        )
                    nc.scalar.activation(
                        a2_t[:, mt, :], ps2[:],
                        mybir.ActivationFunctionType.Relu,
                        bias=b1s[:, mt:mt + 1],
                    )
                for sub in range(2):
                    nt = ch * 2 + sub
                    psn_t = ps_n.tile([128, 64], F32)
                    ps3 = psn_t
                    for k in range(2):
                        nc.tensor.matmul(
                            ps3[:],
                            lhsT=a2_t[:, k, sub * 128:(sub + 1) * 128],
                            rhs=w2s[:, k, :],
                            start=(k == 0), stop=(k == 1),
                        )
                    nc.vector.tensor_add(h_cur[:, nt, :], ps3[:], b2s[:])
                    nc.vector.tensor_scalar_mul(
                        h0a[:, nt, :], h_cur[:, nt, :], ALPHA)
                    nc.scalar.activation(
                        hs[:, nt, :], h_cur[:, nt, :],
                        mybir.ActivationFunctionType.Identity,
                        scale=nss[:, nt:nt + 1],
                    )
                if ch == mlp_a_chunk:
                    emit_cc(0, 0)  # table A collective overlaps MLP tail
            emit_cc(1, 0)

            # ---- propagation ----
            for step in range(K):
                pp = step % 2
                for ci, (b0, b1) in enumerate(chunks):
                    ch_col0 = int(col_of[b0, 0])
                    msg_t = msgp.tile([128, max_chunk_cols, 64], BF)
                    if skip_gather:
                        nc.vector.memset(msg_t[:], 0.0)
                    for kap in range(4):
                        _, c0, ncol, w0 = call_meta[ci * 4 + kap]
                        if ncol == 0 or skip_gather:
                            continue
                        buf, half = kap // 2, kap % 2
                        hb = hbA[pp] if buf == 0 else hbB[pp]
                        # single_packet packs 64 descs/engine -> <=1024 idx
                        for sc in range(0, ncol, gcall):
                            sn = min(gcall, ncol - sc)
                            o0 = c0 - ch_col0 + sc
                            dma_gather_p(
                                nc.gpsimd,
                                out_ap=msg_t[:, o0:o0 + sn, :],
                                in_ap=hb[:, half * 64:(half + 1) * 64],
                                idxs_ap=idxs[:, w0 + sc * 8:w0 + (sc + sn) * 8],
                                num_idxs=sn * 128,
                                num_idxs_reg=sn * 128,
                                elem_size=64,
                                elem_step=128,
                                single_packet=True,
                                queue_num=gq_ctr[0] % gqueues,
                            )
                            gq_ctr[0] += 1
                    for nt in range(b0, b1):
                        ntk = int(TK[nt].sum())
                        if ntk == 0:
                            nc.vector.tensor_scalar_mul(
                                h_cur[:, nt, :], h0a[:, nt, :], 1.0)
                        else:
                            psn_t = ps_n.tile([128, 64], F32)
                            ps = psn_t
                            done = 0
                            for kap in range(4):
                                tkk = int(TK[nt, kap])
                                if tkk == 0:
                                    continue
                                cb = int(col_of[nt, kap])
                                s_t = spool.tile([128, MAXTK, 128], BF)
                                ia = ios[:]
                                io_b = bass.AP(
                                    ia.tensor, ia.offset,
                                    [list(ia.ap[0]), [0, tkk],
                                     list(ia.ap[1])])
                                da = dls[:, cb:cb + tkk]
                                dl_b = bass.AP(
                                    da.tensor, da.offset,
                                    [list(da.ap[0]), list(da.ap[1]),
                                     [0, 128]])
                                nc.vector.tensor_tensor(
                                    s_t[:, 0:tkk, :], io_b, dl_b,
                                    AluOpType.is_equal)
                                for g in range(tkk):
                                    col = cb + g
                                    if not skip_mm or done == 0 or done == ntk - 1:
                                        nc.tensor.matmul(
                                            ps[:],
                                            lhsT=s_t[:, g, :],
                                            rhs=msg_t[:, col - ch_col0, :],
                                            start=(done == 0),
                                            stop=(done == ntk - 1),
                                        )
                                    done += 1
                            nc.vector.scalar_tensor_tensor(
                                h_cur[:, nt, :], ps[:], nds[:, nt:nt + 1],
                                h0a[:, nt, :],
                                AluOpType.mult, AluOpType.add,
                            )
                        if step < K - 1:
                            nc.scalar.activation(
                                hs[:, nt, :], h_cur[:, nt, :],
                                mybir.ActivationFunctionType.Identity,
                                scale=nss[:, nt:nt + 1],
                            )
                    # CC_A one chunk after its hs banks complete: its input
                    # DMA finishes while that chunk's gathers drain, so the
                    # collective never stalls the gpsimd queue head (gathers
                    # for later chunks sit behind it in program order).
                    if step < K - 1:
                        if ci == cc_a_chunk + 1:
                            emit_cc(0, 1 - pp)
                        if ci == len(chunks) - 1:
                            emit_cc(1, 1 - pp)
            nc.sync.dma_start(out[:], h_cur[:])

    nc.compile()
    return nc


# ---------------------------------------------------------------------------
# host wrapper
# ---------------------------------------------------------------------------

def _prep_in_maps(features, W0, b0, W1, b1, W2, b2, pre):
    in_maps = []
    w0t = np.ascontiguousarray(
        W0.T.astype(BF16).reshape(4, 128, 512).transpose(1, 0, 2))
    w1t = np.ascontiguousarray(
        W1.T.astype(BF16).reshape(4, 128, 256).transpose(1, 0, 2))
    w2t = np.ascontiguousarray(
        W2.T.astype(BF16).reshape(2, 128, 64).transpose(1, 0, 2))
    b0cc = np.ascontiguousarray(b0.astype(np.float32).reshape(4, 128).T)
    b1cc = np.ascontiguousarray(b1.astype(np.float32).reshape(2, 128).T)
    b2rr = np.ascontiguousarray(
        np.tile(b2.astype(np.float32)[None, :], (128, 1)))
    X = features.astype(np.float32)
    for c in range(NCORES):
        nodes = np.arange(c * SH_N, (c + 1) * SH_N)
        xt_c = np.zeros((128, 4, SH), BF16)
        Xc = X[nodes].astype(BF16)
        xt_full = Xc.T.reshape(4, 128, SH_N).transpose(1, 0, 2)
        xt_c[:, :, :SH_N] = xt_full
        in_maps.append(dict(
            xt=xt_c, w0t=w0t, w1t=w1t, w2t=w2t,
            b0c=b0cc, b1c=b1cc, b2r=b2rr,
            nsb=pre["ns_arrs"][c], ndb=pre["nd_arrs"][c],
            iota=pre["iota"], idxw=pre["idxw_arrs"][c],
            dlane=pre["dl_arrs"][c],
        ))
    return in_maps


_CACHE = {}


def _get_compiled(edge_index):
    key = hash(np.asarray(edge_index).tobytes())
    if key not in _CACHE:
        pre = preprocess(edge_index)
        nc = build(pre["meta"], gqueues=GQUEUES)
        _CACHE[key] = (pre, nc)
    return _CACHE[key]


def kernel(features, edge_index, W0, b0, W1, b1, W2, b2, _trace=False):
    pre, nc = _get_compiled(edge_index)
    in_maps = _prep_in_maps(features, W0, b0, W1, b1, W2, b2, pre)
    res = run_bass_kernel_spmd(
        nc, in_maps, core_ids=list(range(NCORES)), trace=_trace)
    kernel.last_result = res
    out = np.empty((N, C), np.float32)
    p_of, t_of = pre["p_of"], pre["t_of"]
    for c in range(NCORES):
        nodes = np.arange(c * SH_N, (c + 1) * SH_N)
        oc = np.asarray(res.results[c]["out"]).reshape(128, NTILES, C)
        out[nodes] = oc[p_of[nodes], t_of[nodes]]
    return out



# revision 25
# speedup vs baseline: 1.0032x; 1.0032x over previous
"""APPNP GNN kernel for 8 Trainium2 NeuronCores.

Per core (nodes sharded, 12672 lanes incl. pads):
  MLP: X @ W0.T -> relu -> @ W1.T -> relu -> @ W2.T  (bf16 matmuls, f32 psum)
  K=10 propagation steps:
    hs = h * norm_src                        (fp8 e4m3, [128, NT, 64])
    one AllGather hs -> quad-table [NPAD/4, 256] fp8 (Shared output, one
      buffer per step so the collective takes the fast peer-write path)
    dma_gather per (chunk, quarter): 64B sub-rows from the 256B-strided table
    one-hot S tiles (DVE is_equal vs iota, bf16) + S-stationary matmul
      (bf16 x fp8 -> f32 psum) segment-sum
    h = (1-a)*norm_dst*agg + a*h0
"""
import sys

sys.path.insert(0, "/opt/trn_rl_repo")

import numpy as np
import ml_dtypes

import inspect
import textwrap

import concourse.bass as bass
import concourse.bacc as bacc
import concourse.tile as tile
import concourse.mybir as mybir
from concourse.bass_utils import run_bass_kernel_spmd
from concourse.alu_op_type import AluOpType

BF16 = ml_dtypes.bfloat16
F32 = mybir.dt.float32
BF = mybir.dt.bfloat16
F8 = mybir.dt.float8e4
I16 = mybir.dt.int16

# problem constants
N = 100000
E = 1000000
IN = 512
C = 64
K = 10
ALPHA = 0.1

NCORES = 8
SH_N = N // NCORES              # real nodes per core
NTILES = (SH_N + 127) // 128 + 1  # node tiles per core (+1 all-pad tile)
SH = NTILES * 128               # padded nodes per core
NPAD = NCORES * SH
GQUEUES = 4                     # SWDGE queues for gather round-robin
CHUNK_BANKS = 16                # banks per gather chunk
PAD_LANE_VAL = 200.0            # dst-lane sentinel for pad slots

# single fp8 table: 4 consecutive slots packed per 256B row so quad ids fit
# the gather's int16 index. Edge class = src slot % 4 (which 64B quarter).
QROWS = NPAD // 4                  # quad rows in the table
assert QROWS < 32768


def _install_dma_gather_patched():
    """Clone bass dma_gather with the 256B elem-size assert relaxed to 128B.

    The underlying ucode supports 128B transfers over a 256B-strided table;
    only the bass-side assert (written for the transpose path) blocks it.
    """
    if hasattr(bass.BassGpSimd, "dma_gather_patched"):
        return bass.BassGpSimd.dma_gather_patched
    src = inspect.getsource(bass.BassGpSimd.dma_gather)
    src = textwrap.dedent(src)
    src = src.replace(
        "elem_size_bytes > 0 and elem_size_bytes % 256 == 0",
        "elem_size_bytes > 0 and elem_size_bytes % 64 == 0",
    )
    src = src.replace("def dma_gather(", "def dma_gather_patched(")
    ns = dict(bass.BassGpSimd.dma_gather.__globals__)
    exec(compile(src, "<dma_gather_patched>", "exec"), ns)
    fn = ns["dma_gather_patched"]
    bass.BassGpSimd.dma_gather_patched = fn
    return fn


# ---------------------------------------------------------------------------
# host-side graph preprocessing
# ---------------------------------------------------------------------------

def preprocess(edge_index):
    src = np.asarray(edge_index[0], dtype=np.int64)
    dst = np.asarray(edge_index[1], dtype=np.int64)
    ne = src.shape[0]
    deg_out = np.bincount(src, minlength=N).astype(np.float32)
    deg_in = np.bincount(dst, minlength=N).astype(np.float32)
    ns_full = np.maximum(deg_out, 1.0) ** -0.5
    nd_full = np.maximum(deg_in, 1.0) ** -0.5

    core_of = np.arange(N) // SH_N
    lane_of = np.arange(N) % SH_N          # natural order within core
    p_of = lane_of % 128
    t_of = lane_of // 128
    # global table slot (matches cin [128, NTILES, 64] layout, cores concat)
    slot_of = core_of * (128 * NTILES) + p_of * NTILES + t_of

    nb = NTILES - 1  # active banks

    # edge -> (core, bank, class); class = src slot % 4 (quad-row quarter)
    c_e = core_of[dst]
    nt_e = t_of[dst]
    slot_s = slot_of[src]
    kappa_e = slot_s % 4  # class 0..3

    # counts per (core, bank, class)
    key = ((c_e * nb + nt_e) * 4 + kappa_e).astype(np.int64)
    cnt = np.bincount(key, minlength=NCORES * nb * 4).reshape(NCORES, nb, 4)
    tk = -(-cnt // 128)              # tiles per (core, bank, class)
    TK = tk.max(axis=0)              # common schedule [nb, 4]

    # chunk structure over banks
    chunk_starts = list(range(0, nb, CHUNK_BANKS))
    chunks = [(s, min(s + CHUNK_BANKS, nb)) for s in chunk_starts]

    # column layout: for chunk: for kappa: for nt in chunk: TK[nt, kappa] tiles
    col_of = np.zeros((nb, 4), np.int64)   # first col of (nt, kappa)
    call_cols = []                          # per (chunk, kappa): (col0, ncols)
    col = 0
    for (b0, b1) in chunks:
        for kap in range(4):
            c0 = col
            for nt in range(b0, b1):
                col_of[nt, kap] = col
                col += TK[nt, kap]
            call_cols.append((c0, col - c0))
    ncols = col

    # pad target rows per class: pad slots carry dl=PAD_LANE_VAL, so their
    # S column is all-zero and the (finite, real) gathered row contributes
    # nothing -- any in-range row works; use quad row 0.
    pad_quad_local = np.zeros(4, np.int64)

    # edge sort: by (core, bank, class)
    order_e = np.argsort(key, kind="stable")
    key_s = key[order_e]
    starts = np.concatenate([[0], np.cumsum(np.bincount(
        key_s, minlength=NCORES * nb * 4))])
    pos_e = np.arange(ne) - starts[key_s]

    src_sorted = src[order_e]
    dst_sorted = dst[order_e]
    c_s = c_e[order_e]
    nt_s = nt_e[order_e]
    kap_s = kappa_e[order_e]
    quad_local_s = slot_of[src_sorted] // 4
    assert quad_local_s.max() < 32768
    lane_s = p_of[dst_sorted].astype(np.float32)

    gcol_s = col_of[nt_s, kap_s] + pos_e // 128
    gp_s = pos_e % 128

    colclass = np.empty(ncols, np.int64)
    for nt in range(nb):
        for kap in range(4):
            colclass[col_of[nt, kap]:col_of[nt, kap] + TK[nt, kap]] = kap

    idx_arrs = []   # [128, ncols] int16 (slot s at partition s)
    dl_arrs = []    # [128, ncols] bf16 dst-lane per slot
    for c in range(NCORES):
        m = c_s == c
        idx_a = np.empty((128, ncols), np.int16)
        dl_a = np.full((128, ncols), PAD_LANE_VAL, np.float32)
        idx_a[:] = pad_quad_local[colclass][None, :].astype(np.int16)
        idx_a[gp_s[m], gcol_s[m]] = quad_local_s[m].astype(np.int16)
        dl_a[gp_s[m], gcol_s[m]] = lane_s[m]
        idx_arrs.append(idx_a)
        dl_arrs.append(dl_a.astype(BF16))

    # wrapped gather index inputs: per call block [128, n_call/16]
    call_meta = []  # (kappa, col0, ncols_call, wrap_col0)
    wrap_col = 0
    for ci, (b0, b1) in enumerate(chunks):
        for kap in range(4):
            c0, ncol = call_cols[ci * 4 + kap]
            call_meta.append((kap, c0, ncol, wrap_col))
            wrap_col += ncol * 8  # 128 idx per col -> 8 wrap-cols
    totw = wrap_col
    idxw_arrs = []
    for c in range(NCORES):
        w = np.zeros((128, totw), np.int16)
        for (kap, c0, ncol, w0) in call_meta:
            if ncol == 0:
                continue
            blk = idx_arrs[c][:, c0:c0 + ncol]          # [128, ncol]
            flat = blk.T.reshape(-1)                    # slot order (col-major)
            wrapped = flat.reshape(-1, 16).T            # [16, ncol*8]
            w[:, w0:w0 + ncol * 8] = np.tile(wrapped, (8, 1))
        idxw_arrs.append(w)

    # norms [128, NTILES]
    ns_arrs, nd_arrs = [], []
    for c in range(NCORES):
        ns_a = np.zeros((128, NTILES), np.float32)
        nd_a = np.zeros((128, NTILES), np.float32)
        nodes = np.arange(c * SH_N, (c + 1) * SH_N)
        ns_a[p_of[nodes], t_of[nodes]] = ns_full[nodes]
        nd_a[p_of[nodes], t_of[nodes]] = (1.0 - ALPHA) * nd_full[nodes]
        ns_arrs.append(ns_a)
        nd_arrs.append(nd_a)

    iota = np.tile(
        np.arange(128, dtype=np.float32)[None, :], (128, 1)).astype(BF16)

    meta = dict(TK=TK, col_of=col_of, chunks=chunks, call_meta=call_meta,
                ncols=ncols, totw=totw)
    return dict(meta=meta, idxw_arrs=idxw_arrs, dl_arrs=dl_arrs,
                ns_arrs=ns_arrs, nd_arrs=nd_arrs, iota=iota,
                core_of=core_of, p_of=p_of, t_of=t_of)


# ---------------------------------------------------------------------------
# device graph builder
# ---------------------------------------------------------------------------

def build(meta, skip_cc=False, skip_gather=False, skip_mm=False, gqueues=None,
          gather_elem=None, gcall=16, cc_shared=True, gsp=False):
    if gqueues is None:
        gqueues = GQUEUES
    """gather_elem: timing-only experiment — memset msg tiles (as skip_gather)
    and issue gathers of gather_elem elements into a scratch tile instead.
    Results are wrong; only the schedule/timing is meaningful."""
    import os
    dma_gather_p = _install_dma_gather_patched()

    TK = meta["TK"]
    col_of = meta["col_of"]
    chunks = meta["chunks"]
    call_meta = meta["call_meta"]
    ncols = meta["ncols"]
    totw = meta["totw"]
    nb = NTILES - 1

    nc = bacc.Bacc("TRN2", target_bir_lowering=False, debug=False,
                   num_devices=NCORES, num_swdge_queues=gqueues)

    xt = nc.dram_tensor("xt", [128, 4, SH], BF, kind="ExternalInput")
    w0t = nc.dram_tensor("w0t", [128, 4, 512], BF, kind="ExternalInput")
    w1t = nc.dram_tensor("w1t", [128, 4, 256], BF, kind="ExternalInput")
    w2t = nc.dram_tensor("w2t", [128, 2, 64], BF, kind="ExternalInput")
    b0c = nc.dram_tensor("b0c", [128, 4], F32, kind="ExternalInput")
    b1c = nc.dram_tensor("b1c", [128, 2], F32, kind="ExternalInput")
    b2r = nc.dram_tensor("b2r", [128, 64], F32, kind="ExternalInput")
    nsb = nc.dram_tensor("nsb", [128, NTILES], F32, kind="ExternalInput")
    ndb = nc.dram_tensor("ndb", [128, NTILES], F32, kind="ExternalInput")
    iod = nc.dram_tensor("iota", [128, 128], BF, kind="ExternalInput")
    idxd = nc.dram_tensor("idxw", [128, totw], I16, kind="ExternalInput")
    dld = nc.dram_tensor("dlane", [128, ncols], BF, kind="ExternalInput")
    out = nc.dram_tensor("out", [128, NTILES, 64], F32, kind="ExternalOutput")

    MAXTK = int(TK.max())
    max_chunk_cols = max(
        int(sum(TK[nt, kap] for nt in range(b0, b1) for kap in range(4)))
        for (b0, b1) in chunks)

    with tile.TileContext(nc) as tc:
        with (
            tc.tile_pool(name="const", bufs=1) as constp,
            tc.tile_pool(name="state", bufs=1) as statep,
            tc.tile_pool(name="xtp", bufs=3) as xtp,
            tc.tile_pool(name="a1p", bufs=2) as a1p,
            tc.tile_pool(name="a2p", bufs=2) as a2p,
            tc.tile_pool(name="msgp", bufs=2) as msgp,
            tc.tile_pool(name="sp", bufs=3) as spool,
            tc.tile_pool(name="psw", bufs=2, space="PSUM") as ps_w,
            tc.tile_pool(name="psn", bufs=4, space="PSUM") as ps_n,
            tc.tile_pool(name="dram", bufs=1, space="DRAM") as dramp,
        ):
            # ---- constants ----
            w0s = constp.tile([128, 4, 512], BF)
            w1s = constp.tile([128, 4, 256], BF)
            w2s = constp.tile([128, 2, 64], BF)
            b0s = constp.tile([128, 4], F32)
            b1s = constp.tile([128, 2], F32)
            b2s = constp.tile([128, 64], F32)
            nss = constp.tile([128, NTILES], F32)
            nds = constp.tile([128, NTILES], F32)
            ios = constp.tile([128, 128], BF)
            idxs = constp.tile([128, totw], I16)
            dls = constp.tile([128, ncols], BF)
            for dst_t, src_t in [(w0s, w0t), (w1s, w1t), (w2s, w2t),
                                 (b0s, b0c), (b1s, b1c), (b2s, b2r),
                                 (nss, nsb), (nds, ndb), (ios, iod),
                                 (idxs, idxd), (dls, dld)]:
                nc.sync.dma_start(dst_t[:], src_t[:])

            h_cur = statep.tile([128, NTILES, 64], F32)
            h0a = statep.tile([128, NTILES, 64], F32)
            hs = statep.tile([128, NTILES, 64], F8)

            # pad bank: zero once
            nc.vector.memset(h_cur[:, nb, :], 0.0)
            nc.vector.memset(h0a[:, nb, :], 0.0)
            nc.vector.memset(hs[:, nb, :], 0.0)

            # collective: one AllGather per step into a Shared quad-table.
            # Inputs double-buffered; Shared outputs need a single writer
            # inst, so one table buffer per step.
            ccb = [dramp.tile([128, NTILES, 64], F8, name=f"ccb{i}")
                   for i in range(2)]
            hb_space = "Shared" if cc_shared else "Local"
            nhb = max(K, 1) if cc_shared else 2
            hbt = [dramp.tile([QROWS, 256], F8, name=f"hbt{i}",
                              addr_space=hb_space)
                   for i in range(nhb)]
            gq_ctr = [0]

            def emit_cc(step):
                cin = ccb[step % 2]
                hb = hbt[step % nhb]
                nc.sync.dma_start(cin[:], hs[:])
                if not skip_cc:
                    nc.gpsimd.collective_compute(
                        "AllGather",
                        mybir.AluOpType.bypass,
                        replica_groups=[list(range(NCORES))],
                        ins=[cin.opt()],
                        outs=[hb.opt()],
                    )

            # ---- MLP over chunks of 2 node-tiles ----
            for ch in range(nb // 2):
                c0 = ch * 256
                xt_t = xtp.tile([128, 4, 256], BF)
                nc.sync.dma_start(xt_t[:], xt[:, :, c0:c0 + 256])
                a1_t = a1p.tile([128, 4, 256], BF)
                for mt in range(4):
                    psw_t = ps_w.tile([128, 256], F32)
                    ps1 = psw_t
                    for k in range(4):
                        nc.tensor.matmul(
                            ps1[:],
                            lhsT=w0s[:, k, mt * 128:(mt + 1) * 128],
                            rhs=xt_t[:, k, :],
                            start=(k == 0), stop=(k == 3),
                        )
                    nc.scalar.activation(
                        a1_t[:, mt, :], ps1[:],
                        mybir.ActivationFunctionType.Relu,
                        bias=b0s[:, mt:mt + 1],
                    )
                a2_t = a2p.tile([128, 2, 256], BF)
                for mt in range(2):
                    psw_t = ps_w.tile([128, 256], F32)
                    ps2 = psw_t
                    for k in range(4):
                        nc.tensor.matmul(
                            ps2[:],
                            lhsT=w1s[:, k, mt * 128:(mt + 1) * 128],
                            rhs=a1_t[:, k, :],
                            start=(k == 0), stop=(k == 3),
                        )
                    nc.scalar.activation(
                        a2_t[:, mt, :], ps2[:],
                        mybir.ActivationFunctionType.Relu,
                        bias=b1s[:, mt:mt + 1],
                    )
                for sub in range(2):
                    nt = ch * 2 + sub
                    psn_t = ps_n.tile([128, 64], F32)
                    ps3 = psn_t
                    for k in range(2):
                        nc.tensor.matmul(
                            ps3[:],
                            lhsT=a2_t[:, k, sub * 128:(sub + 1) * 128],
                            rhs=w2s[:, k, :],
                            start=(k == 0), stop=(k == 1),
                        )
                    nc.vector.tensor_add(h_cur[:, nt, :], ps3[:], b2s[:])
                    nc.vector.tensor_scalar_mul(
                        h0a[:, nt, :], h_cur[:, nt, :], ALPHA)
                    nc.scalar.activation(
                        hs[:, nt, :], h_cur[:, nt, :],
                        mybir.ActivationFunctionType.Identity,
                        scale=nss[:, nt:nt + 1],
                    )
            emit_cc(0)

            # ---- propagation ----
            for step in range(K):
                for ci, (b0, b1) in enumerate(chunks):
                    ch_col0 = int(col_of[b0, 0])
                    msg_t = msgp.tile([128, max_chunk_cols, 64], F8)
                    if skip_gather:
                        nc.vector.memset(msg_t[:], 0.0)
                    for kap in range(4):
                        _, c0, ncol, w0 = call_meta[ci * 4 + kap]
                        if ncol == 0 or skip_gather:
                            continue
                        hb = hbt[step % nhb]
                        # single_packet packs 64 descs/engine -> <=1024 idx
                        for sc in range(0, ncol, gcall):
                            sn = min(gcall, ncol - sc)
                            o0 = c0 - ch_col0 + sc
                            dma_gather_p(
                                nc.gpsimd,
                                out_ap=msg_t[:, o0:o0 + sn, :],
                                in_ap=hb[:, kap * 64:(kap + 1) * 64],
                                idxs_ap=idxs[:, w0 + sc * 8:w0 + (sc + sn) * 8],
                                num_idxs=sn * 128,
                                num_idxs_reg=sn * 128,
                                elem_size=64,
                                elem_step=256,
                                single_packet=gsp,
                                queue_num=gq_ctr[0] % gqueues,
                            )
                            gq_ctr[0] += 1
                    for nt in range(b0, b1):
                        ntk = int(TK[nt].sum())
                        if ntk == 0:
                            nc.vector.tensor_scalar_mul(
                                h_cur[:, nt, :], h0a[:, nt, :], 1.0)
                        else:
                            psn_t = ps_n.tile([128, 64], F32)
                            ps = psn_t
                            done = 0
                            for kap in range(4):
                                tkk = int(TK[nt, kap])
                                if tkk == 0:
                                    continue
                                cb = int(col_of[nt, kap])
                                s_t = spool.tile([128, MAXTK, 128], BF)
                                ia = ios[:]
                                io_b = bass.AP(
                                    ia.tensor, ia.offset,
                                    [list(ia.ap[0]), [0, tkk],
                                     list(ia.ap[1])])
                                da = dls[:, cb:cb + tkk]
                                dl_b = bass.AP(
                                    da.tensor, da.offset,
                                    [list(da.ap[0]), list(da.ap[1]),
                                     [0, 128]])
                                nc.vector.tensor_tensor(
                                    s_t[:, 0:tkk, :], io_b, dl_b,
                                    AluOpType.is_equal)
                                for g in range(tkk):
                                    col = cb + g
                                    if not skip_mm or done == 0 or done == ntk - 1:
                                        nc.tensor.matmul(
                                            ps[:],
                                            lhsT=s_t[:, g, :],
                                            rhs=msg_t[:, col - ch_col0, :],
                                            start=(done == 0),
                                            stop=(done == ntk - 1),
                                        )
                                    done += 1
                            nc.vector.scalar_tensor_tensor(
                                h_cur[:, nt, :], ps[:], nds[:, nt:nt + 1],
                                h0a[:, nt, :],
                                AluOpType.mult, AluOpType.add,
                            )
                        if step < K - 1:
                            nc.scalar.activation(
                                hs[:, nt, :], h_cur[:, nt, :],
                                mybir.ActivationFunctionType.Identity,
                                scale=nss[:, nt:nt + 1],
                            )
                    if step < K - 1 and ci == len(chunks) - 1:
                        emit_cc(step + 1)
            nc.sync.dma_start(out[:], h_cur[:])

    nc.compile()
    return nc


# ---------------------------------------------------------------------------
# host wrapper
# ---------------------------------------------------------------------------

def _prep_in_maps(features, W0, b0, W1, b1, W2, b2, pre):
    in_maps = []
    w0t = np.ascontiguousarray(
        W0.T.astype(BF16).reshape(4, 128, 512).transpose(1, 0, 2))
    w1t = np.ascontiguousarray(
        W1.T.astype(BF16).reshape(4, 128, 256).transpose(1, 0, 2))
    w2t = np.ascontiguousarray(
        W2.T.astype(BF16).reshape(2, 128, 64).transpose(1, 0, 2))
    b0cc = np.ascontiguousarray(b0.astype(np.float32).reshape(4, 128).T)
    b1cc = np.ascontiguousarray(b1.astype(np.float32).reshape(2, 128).T)
    b2rr = np.ascontiguousarray(
        np.tile(b2.astype(np.float32)[None, :], (128, 1)))
    X = features.astype(np.float32)
    for c in range(NCORES):
        nodes = np.arange(c * SH_N, (c + 1) * SH_N)
        xt_c = np.zeros((128, 4, SH), BF16)
        Xc = X[nodes].astype(BF16)
        xt_full = Xc.T.reshape(4, 128, SH_N).transpose(1, 0, 2)
        xt_c[:, :, :SH_N] = xt_full
        in_maps.append(dict(
            xt=xt_c, w0t=w0t, w1t=w1t, w2t=w2t,
            b0c=b0cc, b1c=b1cc, b2r=b2rr,
            nsb=pre["ns_arrs"][c], ndb=pre["nd_arrs"][c],
            iota=pre["iota"], idxw=pre["idxw_arrs"][c],
            dlane=pre["dl_arrs"][c],
        ))
    return in_maps


_CACHE = {}


def _get_compiled(edge_index):
    key = hash(np.asarray(edge_index).tobytes())
    if key not in _CACHE:
        pre = preprocess(edge_index)
        nc = build(pre["meta"], gqueues=GQUEUES)
        _CACHE[key] = (pre, nc)
    return _CACHE[key]


def kernel(features, edge_index, W0, b0, W1, b1, W2, b2, _trace=False):
    pre, nc = _get_compiled(edge_index)
    in_maps = _prep_in_maps(features, W0, b0, W1, b1, W2, b2, pre)
    res = run_bass_kernel_spmd(
        nc, in_maps, core_ids=list(range(NCORES)), trace=_trace)
    kernel.last_result = res
    out = np.empty((N, C), np.float32)
    p_of, t_of = pre["p_of"], pre["t_of"]
    for c in range(NCORES):
        nodes = np.arange(c * SH_N, (c + 1) * SH_N)
        oc = np.asarray(res.results[c]["out"]).reshape(128, NTILES, C)
        out[nodes] = oc[p_of[nodes], t_of[nodes]]
    return out

